# revision 1
# baseline (speedup 1.0000x reference)
"""MultiHeadAttention (B=4, S=2048, D=2048, H=16) on 8 TRN2 NeuronCores.

Sharding: core c handles batch b = c//2 and head-half = c%2 (8 heads).
Each core computes Q/K/V projections for its 1024 rows, attention for its
8 heads, and a partial output projection; the host sums the two partials
per batch and un-permutes.

Layout trick: torch's `view(B, H, S, dk)` head split (no transpose) means
head h of batch b lives in rows [128h, 128h+128) of the projection output,
with each row holding 16 consecutive seq positions. Working in permuted
query/key coordinates pi = 128*t + u (s = 16*u + t), every attention
operand is an exact 128x128 tile of either the transposed projection
(R^T, for Q/K) or the natural projection (R, for V). Softmax is
permutation-invariant, and the host un-permutes the final output.

All matmuls run in float32r (fp32 with 10-bit mantissa, full PE speed);
host pre-rounds all external matmul operands. Weights are pre-tiled on
the host for contiguous loads; every transfer >256KB is split across DMA
queues. Projection input stages rotate through one double-buffered pool.
"""
import math
import os
from contextlib import ExitStack

import numpy as np

B, S, D, H = 4, 2048, 2048, 16
DK = D // H            # 128
HPC = H // 2           # heads per core = 8
RPC = HPC * DK         # rows per core = 1024
NC_ = 8                # cores
MC = D // 128          # contraction chunks = 16
SCALE = 1.0 / math.sqrt(DK)

_cache = {}
last_results = None


def _round_f32r(x):
    """Round fp32 to the 10-bit-mantissa grid the PE uses for float32r."""
    x = np.ascontiguousarray(x, dtype=np.float32)
    u = x.view(np.uint32)
    lsb = (u >> np.uint32(13)) & np.uint32(1)
    r = (u + np.uint32(0x0FFF) + lsb) & np.uint32(0xFFFFE000)
    return r.view(np.float32)


def _build():
    import concourse.bass as bass
    import concourse.mybir as mybir
    import concourse.tile as tile
    from concourse import bacc

    f32 = mybir.dt.float32
    f32r = mybir.dt.float32r
    AF = mybir.ActivationFunctionType

    nc = bacc.Bacc("TRN2", target_bir_lowering=False, debug=False,
                   num_devices=NC_)

    # ---- external I/O ----
    qts_d = nc.dram_tensor("qts", (MC, 128, RPC), f32r, kind="ExternalInput")
    kts_d = nc.dram_tensor("kts", (MC, 128, RPC), f32r, kind="ExternalInput")
    vts_d = nc.dram_tensor("vts", (MC, 128, RPC), f32r, kind="ExternalInput")
    wqt_d = nc.dram_tensor("wqt", (MC, 128, MC, 128), f32r, kind="ExternalInput")
    wkt_d = nc.dram_tensor("wkt", (MC, 128, MC, 128), f32r, kind="ExternalInput")
    wvt_d = nc.dram_tensor("wvt", (8, 128, MC, 256), f32r, kind="ExternalInput")
    wot_d = nc.dram_tensor("wot", (MC, 128, HPC, 128), f32r, kind="ExternalInput")
    bqs_d = nc.dram_tensor("bqs", (D,), f32, kind="ExternalInput")
    bk_d = nc.dram_tensor("bk", (D,), f32, kind="ExternalInput")
    bvr_d = nc.dram_tensor("bvr", (1, D), f32r, kind="ExternalInput")
    bo_d = nc.dram_tensor("bo", (D,), f32, kind="ExternalInput")
    ones1_d = nc.dram_tensor("ones1", (1, 128), f32r, kind="ExternalInput")
    onescol_d = nc.dram_tensor("onescol", (128, 1), f32r, kind="ExternalInput")
    out_d = nc.dram_tensor("out", (D, S), f32, kind="ExternalOutput")

    with tile.TileContext(nc) as tc, ExitStack() as top:
        rpool = top.enter_context(tc.tile_pool(name="consts", bufs=1))
        dpool = top.enter_context(tc.tile_pool(name="dram", bufs=1, space="DRAM"))

        bq_sb = rpool.tile([128, MC], f32)
        bk_sb = rpool.tile([128, MC], f32)
        bo_sb = rpool.tile([128, MC], f32)
        bv_sb = rpool.tile([1, D], f32r)
        ones1 = rpool.tile([1, 128], f32r)
        onescol = rpool.tile([128, 1], f32r)
        nc.sync.dma_start(bq_sb[:], bqs_d.ap().rearrange("(t p) -> p t", p=128))
        nc.sync.dma_start(bk_sb[:], bk_d.ap().rearrange("(t p) -> p t", p=128))
        nc.sync.dma_start(bo_sb[:], bo_d.ap().rearrange("(t p) -> p t", p=128))
        nc.sync.dma_start(bv_sb[:], bvr_d.ap())
        nc.sync.dma_start(ones1[:], ones1_d.ap())
        nc.sync.dma_start(onescol[:], onescol_d.ap())

        qhat_dram = dpool.tile([MC, 128, RPC], f32r)        # [t][dk][r]
        khat_dram = dpool.tile([HPC, 128, MC, 128], f32r)   # [h][dk][tk][u]
        vhat_dram = dpool.tile([RPC, D], f32r)              # natural R_v

        def load_stage(pool, src_d):
            st = pool.tile([128, MC, RPC], f32r, tag="st")
            for mc in range(MC):
                nc.sync.dma_start(st[:, mc, :], src_d.ap()[mc])
            return st

        def load_w16(pool, src_ap, tag):
            """Load a [128, MC, 128] weight tile in 4 mc-chunks."""
            st = pool.tile([128, MC, 128], f32r, tag=tag)
            for g in range(4):
                nc.sync.dma_start(st[:, 4 * g:4 * g + 4, :],
                                  src_ap[:, 4 * g:4 * g + 4, :])
            return st

        with ExitStack() as stages_es:
            st_pool = stages_es.enter_context(tc.tile_pool(name="stages", bufs=2))
            w_pool = stages_es.enter_context(tc.tile_pool(name="weights", bufs=3))

            # ============= phase Q (first; weights load from t=0) ======
            with ExitStack() as ph:
                qps_pool = ph.enter_context(
                    tc.tile_pool(name="qps", bufs=4, space="PSUM"))
                qout_pool = ph.enter_context(tc.tile_pool(name="qout", bufs=4))
                with nc.named_scope("proj_q"):
                    qt_st = load_stage(st_pool, qts_d)
                    vt_st = load_stage(st_pool, vts_d)   # prefetch for V
                    for ct in range(MC):
                        wq_st = load_w16(w_pool, wqt_d.ap()[ct], "w")
                        for rb in range(2):
                            ps = qps_pool.tile([128, 512], f32, tag="qps")
                            for mc in range(MC):
                                nc.tensor.matmul(
                                    ps[:], wq_st[:, mc, :],
                                    qt_st[:, mc, 512 * rb:512 * rb + 512],
                                    start=(mc == 0), stop=(mc == MC - 1))
                            qo = qout_pool.tile([128, 512], f32r, tag="qo")
                            nc.scalar.activation(qo[:], ps[:], AF.Identity,
                                                 bias=bq_sb[:, ct:ct + 1],
                                                 scale=SCALE)
                            nc.gpsimd.dma_start(
                                qhat_dram[ct, :, 512 * rb:512 * rb + 512],
                                qo[:])

            # ============= phase V (+ prefetch K stage) ================
            with ExitStack() as ph:
                vps_pool = ph.enter_context(
                    tc.tile_pool(name="vps", bufs=4, space="PSUM"))
                vout_pool = ph.enter_context(tc.tile_pool(name="vout", bufs=4))
                with nc.named_scope("proj_v"):
                    kt_st = load_stage(st_pool, kts_d)   # prefetch (rotates)
                    for cb in range(8):           # c blocks of 256
                        wv_st = w_pool.tile([128, MC, 256], f32r, tag="w")
                        for g in range(4):
                            nc.sync.dma_start(
                                wv_st[:, 4 * g:4 * g + 4, :],
                                wvt_d.ap()[cb][:, 4 * g:4 * g + 4, :])
                        for rt in range(8):       # r tiles of 128
                            ps = vps_pool.tile([128, 256], f32, tag="vps")
                            for mc in range(MC):
                                nc.tensor.matmul(
                                    ps[:],
                                    vt_st[:, mc, 128 * rt:128 * rt + 128],
                                    wv_st[:, mc, :], start=(mc == 0),
                                    stop=False)
                            nc.tensor.matmul(
                                ps[:], ones1[:],
                                bv_sb[:, 256 * cb:256 * cb + 256],
                                start=False, stop=True)
                            vo = vout_pool.tile([128, 256], f32r, tag="vo")
                            nc.vector.tensor_copy(vo[:], ps[:])
                            nc.gpsimd.dma_start(
                                vhat_dram[128 * rt:128 * rt + 128,
                                          256 * cb:256 * cb + 256], vo[:])

            # ============= phase K -> khat_dram [h][dk][tk][u] =========
            with ExitStack() as ph:
                kps_pool = ph.enter_context(
                    tc.tile_pool(name="kps", bufs=4, space="PSUM"))
                kout_pool = ph.enter_context(tc.tile_pool(name="kout", bufs=4))
                with nc.named_scope("proj_k"):
                    for ct in range(MC):
                        wk_st = load_w16(w_pool, wkt_d.ap()[ct], "w")
                        for rb in range(2):
                            ps = kps_pool.tile([128, 512], f32, tag="kps")
                            for mc in range(MC):
                                nc.tensor.matmul(
                                    ps[:], wk_st[:, mc, :],
                                    kt_st[:, mc, 512 * rb:512 * rb + 512],
                                    start=(mc == 0), stop=(mc == MC - 1))
                            ko = kout_pool.tile([128, 4, 128], f32r, tag="ko")
                            nc.scalar.activation(ko[:], ps[:], AF.Identity,
                                                 bias=bk_sb[:, ct:ct + 1],
                                                 scale=1.0)
                            dst = khat_dram[4 * rb:4 * rb + 4, :, ct, :] \
                                .rearrange("h p u -> p h u")
                            nc.gpsimd.dma_start(dst, ko[:])

        # ============= attention + output projection ===============
        with ExitStack() as ph:
            q_pool = ph.enter_context(tc.tile_pool(name="qrhs", bufs=4))
            kh_pool = ph.enter_context(tc.tile_pool(name="kh", bufs=3))
            vh_pool = ph.enter_context(tc.tile_pool(name="vh", bufs=3))
            exp_pool = ph.enter_context(tc.tile_pool(name="expp", bufs=6))
            tree_pool = ph.enter_context(tc.tile_pool(name="tree", bufs=2))
            scps_pool = ph.enter_context(
                tc.tile_pool(name="scps", bufs=2, space="PSUM"))
            xps_pool = ph.enter_context(
                tc.tile_pool(name="xps", bufs=2, space="PSUM"))
            sps_pool = ph.enter_context(
                tc.tile_pool(name="sps", bufs=1, space="PSUM"))
            ops_pool = ph.enter_context(
                tc.tile_pool(name="ops", bufs=1, space="PSUM"))
            nrm_pool = ph.enter_context(tc.tile_pool(name="nrm", bufs=2))
            x_pool = ph.enter_context(tc.tile_pool(name="xsb", bufs=4))
            wo_pool = ph.enter_context(tc.tile_pool(name="wo", bufs=3))
            oout_pool = ph.enter_context(tc.tile_pool(name="oout", bufs=3))
            with nc.named_scope("attn"):
                NP_ = MC // 2     # key-tile pairs per head

                def attn_block(j, h, x_j, k_h, v_h, defer_in):
                    q_rhs = q_pool.tile([128, 4, 128], f32r, tag="qr")
                    nc.sync.dma_start(
                        q_rhs[:],
                        qhat_dram[4 * j:4 * j + 4, :, 128 * h:128 * h + 128]
                        .rearrange("t p u -> p t u"))
                    x_ps = xps_pool.tile([128, 512], f32, tag="xps")
                    s_ps = sps_pool.tile([1, 512], f32, tag="sps")
                    acc = tree_pool.tile([128, 2, 512], f32, tag="acc")
                    tsum = tree_pool.tile([128, 512], f32r, tag="tf")
                    exs = [None] * NP_

                    ys = [None] * 4
                    zs = [None] * 2

                    def pv_and_sum(tp):
                        ex = exs[tp]
                        for i in range(2):
                            nc.tensor.matmul(
                                x_ps[:], v_h[:, 2 * tp + i, :],
                                ex[:, i, :], start=(tp == 0 and i == 0),
                                stop=(tp == NP_ - 1 and i == 1))
                        if tp % 2 == 1:
                            a = tp // 2
                            y = tree_pool.tile([128, 2, 512], f32, tag="y")
                            nc.vector.tensor_add(
                                y[:], exs[tp - 1][:].bitcast(f32),
                                ex[:].bitcast(f32))
                            ys[a] = y
                        if tp == 3:
                            z = tree_pool.tile([128, 2, 512], f32, tag="z")
                            nc.vector.tensor_add(z[:], ys[0][:], ys[1][:])
                            zs[0] = z
                        elif tp == NP_ - 1:
                            z = tree_pool.tile([128, 2, 512], f32, tag="z")
                            nc.vector.tensor_add(z[:], ys[2][:], ys[3][:])
                            zs[1] = z
                            nc.vector.tensor_add(acc[:, :, :], zs[0][:],
                                                 zs[1][:])
                            nc.vector.tensor_add(tsum[:], acc[:, 0, :],
                                                 acc[:, 1, :])

                    def finisher():
                        nc.tensor.matmul(s_ps[:], onescol[:], tsum[:],
                                         start=True, stop=True)
                        rec = nrm_pool.tile([1, 512], f32, tag="rec")
                        nc.vector.reciprocal_approx_fast(rec[:], s_ps[:])
                        bcast = nrm_pool.tile([128, 512], f32, tag="bc")
                        nc.gpsimd.partition_broadcast(bcast[:], rec[:])
                        nc.vector.tensor_mul(x_j[:, h, :], x_ps[:], bcast[:])

                    for tp in range(NP_):
                        sc = scps_pool.tile([128, 2, 512], f32, tag="sc")
                        for i in range(2):
                            tk = 2 * tp + i
                            nc.tensor.matmul(
                                sc[:, i, :], k_h[:, tk, :],
                                q_rhs[:, :, :], start=True, stop=True)
                        ex = exp_pool.tile([128, 2, 512], f32r, tag="ex")
                        nc.scalar.activation(ex[:], sc[:], AF.Exp, scale=1.0)
                        exs[tp] = ex
                        if tp == 0:
                            for fn in defer_in:
                                fn()
                        if tp >= 2:
                            pv_and_sum(tp - 2)
                    return [lambda: pv_and_sum(NP_ - 2),
                            lambda: pv_and_sum(NP_ - 1), finisher]

                def emit_otile(j, x_j, ot):
                    wo_st = wo_pool.tile([128, HPC, 128], f32r, tag="wo")
                    for g in range(2):
                        nc.sync.dma_start(
                            wo_st[:, 4 * g:4 * g + 4, :],
                            wot_d.ap()[ot][:, 4 * g:4 * g + 4, :])
                    op = ops_pool.tile([128, 512], f32, tag="op")
                    for h in range(HPC):
                        nc.tensor.matmul(op[:], wo_st[:, h, :],
                                         x_j[:, h, :], start=(h == 0),
                                         stop=(h == HPC - 1))
                    oo = oout_pool.tile([128, 512], f32, tag="oo")
                    nc.scalar.activation(oo[:], op[:], AF.Identity,
                                         bias=bo_sb[:, ot:ot + 1],
                                         scale=1.0)
                    nc.gpsimd.dma_start(
                        out_d.ap()[128 * ot:128 * ot + 128,
                                   512 * j:512 * j + 512], oo[:])

                pending = None    # previous pair's (j0, x0, j1, x1)
                defer = []
                for jp in range(2):           # pairs of query pi-blocks
                    j0, j1 = 2 * jp, 2 * jp + 1
                    x_j0 = x_pool.tile([128, HPC, 512], f32r, tag="xj")
                    x_j1 = x_pool.tile([128, HPC, 512], f32r, tag="xj")
                    for h in range(HPC):
                        k_h = kh_pool.tile([128, MC, 128], f32r, tag="kh")
                        for g in range(4):
                            nc.sync.dma_start(
                                k_h[:, 4 * g:4 * g + 4, :],
                                khat_dram[h][:, 4 * g:4 * g + 4, :])
                        v_h = vh_pool.tile([128, MC, 128], f32r, tag="vh")
                        for g in range(4):
                            nc.sync.dma_start(
                                v_h[:, 4 * g:4 * g + 4, :],
                                vhat_dram[128 * h:128 * h + 128,
                                          512 * g:512 * g + 512])
                        defer = attn_block(j0, h, x_j0, k_h, v_h, defer)
                        if pending is not None:
                            pj0, px0, pj1, px1 = pending
                            emit_otile(pj0, px0, 2 * h)
                            emit_otile(pj1, px1, 2 * h)
                        defer = attn_block(j1, h, x_j1, k_h, v_h, defer)
                        if pending is not None:
                            pj0, px0, pj1, px1 = pending
                            emit_otile(pj0, px0, 2 * h + 1)
                            emit_otile(pj1, px1, 2 * h + 1)
                    pending = (j0, x_j0, j1, x_j1)
                # final pair's output projections
                for fn in defer:
                    fn()
                pj0, px0, pj1, px1 = pending
                for ot in range(MC):
                    emit_otile(pj0, px0, ot)
                    emit_otile(pj1, px1, ot)

    nc.compile()
    return nc


def _prep_shared(Wq, Wk, Wv, Wo, bq, bk, bv, bo):
    wqt = _round_f32r(np.ascontiguousarray(np.asarray(Wq, np.float32).T))
    wkt = _round_f32r(np.ascontiguousarray(np.asarray(Wk, np.float32).T))
    wvt = _round_f32r(np.ascontiguousarray(np.asarray(Wv, np.float32).T))
    wqt_t = np.ascontiguousarray(
        wqt.reshape(MC, 128, MC, 128).transpose(2, 1, 0, 3))
    wkt_t = np.ascontiguousarray(
        wkt.reshape(MC, 128, MC, 128).transpose(2, 1, 0, 3))
    wvt_t = np.ascontiguousarray(
        wvt.reshape(MC, 128, 8, 256).transpose(2, 1, 0, 3))
    woT = np.ascontiguousarray(np.asarray(Wo, np.float32).T)
    bqs = (np.asarray(bq, np.float32) * SCALE).copy()
    bk_np = np.asarray(bk, np.float32).copy()
    bvr = _round_f32r(np.asarray(bv, np.float32).reshape(1, D))
    bo_np = np.asarray(bo, np.float32).copy()
    return wqt_t, wkt_t, wvt_t, woT, bqs, bk_np, bvr, bo_np


def kernel(Q, K, V, Wq, bq, Wk, bk, Wv, bv, Wo, bo, num_heads):
    global last_results
    assert int(num_heads) == H

    from concourse.bass_utils import run_bass_kernel_spmd

    if "nc" not in _cache:
        _cache["nc"] = _build()
    nc = _cache["nc"]

    Q = np.asarray(Q, np.float32)
    K = np.asarray(K, np.float32)
    V = np.asarray(V, np.float32)
    wqt_t, wkt_t, wvt_t, woT, bqs, bk_np, bvr, bo_np = _prep_shared(
        Wq, Wk, Wv, Wo, bq, bk, bv, bo)
    ones1 = np.ones((1, 128), np.float32)
    onescol = np.ones((128, 1), np.float32)

    in_maps = []
    for c in range(NC_):
        b, half = divmod(c, 2)
        r0 = RPC * half
        wot_t = np.ascontiguousarray(
            _round_f32r(woT[r0:r0 + RPC, :])
            .reshape(HPC, 128, MC, 128).transpose(2, 1, 0, 3))
        in_maps.append({
            "qts": _round_f32r(Q[b].T[:, r0:r0 + RPC]).reshape(MC, 128, RPC),
            "kts": _round_f32r(K[b].T[:, r0:r0 + RPC]).reshape(MC, 128, RPC),
            "vts": _round_f32r(V[b].T[:, r0:r0 + RPC]).reshape(MC, 128, RPC),
            "wqt": wqt_t, "wkt": wkt_t, "wvt": wvt_t, "wot": wot_t,
            "bqs": bqs, "bk": bk_np, "bvr": bvr, "bo": bo_np,
            "ones1": ones1, "onescol": onescol,
        })

    res = run_bass_kernel_spmd(nc, in_maps, core_ids=list(range(NC_)))
    last_results = res

    out = np.empty((B, S, D), np.float32)
    for b in range(B):
        oT = res.results[2 * b]["out"] + res.results[2 * b + 1]["out"]
        # oT[o, pi], pi = 128*t + u ; s = 16*u + t
        out[b] = oT.reshape(D, 16, 128).transpose(2, 1, 0).reshape(S, D)
    return out



# revision 2
# speedup vs baseline: 1.0117x; 1.0117x over previous
"""MultiHeadAttention (B=4, S=2048, D=2048, H=16) on 8 TRN2 NeuronCores — v2.

Sharding: core c handles batch b = c//2 and head-half = c%2 (8 heads = 1024
local seq rows).  Each core computes Q/K/V projections for its 1024 rows,
attention for its 8 heads, and a partial output projection; the host sums the
two partials per batch and un-permutes.

v2 vs baseline: all matmuls in bf16 (1 cycle/row on the PE, same as f32r) and
ALL intermediates (q/k/v projections, Wo) are SBUF-resident — no DRAM spill
round-trips.  HBM traffic drops from ~193MB to ~48MB per core, removing the
DMA co-bottleneck.  Tree-sum of exp tiles runs in bf16 on DVE (2x/4x modes);
softmax normalization is fused per head-block; output projection emits are
interleaved with the next j-block's attention to keep the PE fed.

Layout trick (unchanged): torch's `view(B, H, S, dk)` head split means head h
of batch b lives in rows [128h, 128h+128) of the projection output; in
permuted coordinates pi = 128*t + u (s = 16*u + t) every attention operand is
an exact 128x128 tile of the transposed (Q/K) or natural (V) projection.
Softmax is permutation-invariant; the host un-permutes the final output.
"""
import math
from contextlib import ExitStack

import numpy as np

B, S, D, H = 4, 2048, 2048, 16
DK = D // H            # 128
HPC = H // 2           # heads per core = 8
RPC = HPC * DK         # rows per core = 1024
NC_ = 8                # cores
MC = D // 128          # contraction chunks = 16
NP_ = MC // 2          # key-tile pairs per head = 8
SCALE = 1.0 / math.sqrt(DK)

_cache = {}
last_results = None


def _build():
    import concourse.mybir as mybir
    import concourse.tile as tile
    from concourse import bacc

    f32 = mybir.dt.float32
    bf16 = mybir.dt.bfloat16
    AF = mybir.ActivationFunctionType

    nc = bacc.Bacc("TRN2", target_bir_lowering=False, debug=False,
                   num_devices=NC_)

    # ---- external I/O ----
    qs_d = nc.dram_tensor("qs", (MC, 128, RPC), bf16, kind="ExternalInput")
    ks_d = nc.dram_tensor("ks", (MC, 128, RPC), bf16, kind="ExternalInput")
    vs_d = nc.dram_tensor("vs", (MC, 128, RPC), bf16, kind="ExternalInput")
    wqt_d = nc.dram_tensor("wqt", (MC, 128, MC, 128), bf16, kind="ExternalInput")
    wkt_d = nc.dram_tensor("wkt", (MC, 128, MC, 128), bf16, kind="ExternalInput")
    wvt_d = nc.dram_tensor("wvt", (8, 128, MC, 256), bf16, kind="ExternalInput")
    wot_d = nc.dram_tensor("wot", (128, MC, HPC, 128), bf16, kind="ExternalInput")
    bqs_d = nc.dram_tensor("bqs", (D,), f32, kind="ExternalInput")
    bk_d = nc.dram_tensor("bk", (D,), f32, kind="ExternalInput")
    bvr_d = nc.dram_tensor("bvr", (1, D), bf16, kind="ExternalInput")
    bo_d = nc.dram_tensor("bo", (D,), f32, kind="ExternalInput")
    ones1_d = nc.dram_tensor("ones1", (1, 128), bf16, kind="ExternalInput")
    onescol_d = nc.dram_tensor("onescol", (128, 1), bf16, kind="ExternalInput")
    out_d = nc.dram_tensor("out", (D, S), bf16, kind="ExternalOutput")

    with tile.TileContext(nc) as tc, ExitStack() as top:
        cpool = top.enter_context(tc.tile_pool(name="consts", bufs=1))
        rpool = top.enter_context(tc.tile_pool(name="resident", bufs=1))

        bq_sb = cpool.tile([128, MC], f32, tag="bq")
        bk_sb = cpool.tile([128, MC], f32, tag="bk")
        bo_sb = cpool.tile([128, MC], f32, tag="bo")
        bv_sb = cpool.tile([1, D], bf16, tag="bv")
        ones1 = cpool.tile([1, 128], bf16, tag="o1")
        onescol = cpool.tile([128, 1], bf16, tag="oc")

        def load_consts():
            nc.sync.dma_start(bk_sb[:],
                              bk_d.ap().rearrange("(t p) -> p t", p=128))
            nc.sync.dma_start(bq_sb[:],
                              bqs_d.ap().rearrange("(t p) -> p t", p=128))
            nc.sync.dma_start(bo_sb[:],
                              bo_d.ap().rearrange("(t p) -> p t", p=128))
            nc.sync.dma_start(bv_sb[:], bvr_d.ap())
            nc.sync.dma_start(ones1[:], ones1_d.ap())
            nc.sync.dma_start(onescol[:], onescol_d.ap())

        # SBUF-resident intermediates (bf16):
        #   qhat[dk_p, ct, seq]          transposed Q projection
        #   khat[dk_p, h, tk, u]         transposed K projection per head
        #   vhat[u, h, tv, dk]           natural V projection per head
        qhat = rpool.tile([128, MC, RPC], bf16, tag="qhat")
        khat = rpool.tile([128, HPC, MC, 128], bf16, tag="khat")
        vhat = rpool.tile([128, HPC, MC, 128], bf16, tag="vhat")

        def load_stage(pool, src_d, dma, nchunks=4):
            st = pool.tile([128, MC, RPC], bf16, tag="st")
            w = MC // nchunks
            for g in range(nchunks):
                dma(st[:, w * g:w * g + w, :],
                    src_d.ap()[w * g:w * g + w].rearrange("t p r -> p t r"))
            return st

        def load_w(pool, src_ap):
            st = pool.tile([128, MC, 128], bf16, tag="w")
            for g in range(2):
                nc.sync.dma_start(st[:, 8 * g:8 * g + 8, :],
                                  src_ap[:, 8 * g:8 * g + 8, :])
            return st

        with ExitStack() as proj_es:
            st_pool = proj_es.enter_context(tc.tile_pool(name="stages", bufs=2))
            w_pool = proj_es.enter_context(tc.tile_pool(name="weights", bufs=4))

            # ============= phase K (first: attention needs all of khat) ====
            # DMA issue order matters for the head latency: the first kt
            # chunk and the first two K weight tiles go first, then the rest
            # of kt, then consts and the qt prefetch.
            kt_st = st_pool.tile([128, MC, RPC], bf16, tag="st")

            def kt_chunk(g):
                nc.sync.dma_start(
                    kt_st[:, 2 * g:2 * g + 2, :],
                    ks_d.ap()[2 * g:2 * g + 2].rearrange("t p r -> p t r"))

            kt_chunk(0)
            wk_tiles = {0: load_w(w_pool, wkt_d.ap()[0])}
            kt_chunk(1)
            wk_tiles[1] = load_w(w_pool, wkt_d.ap()[1])
            for g in range(2, 8):
                kt_chunk(g)
            load_consts()
            # qt/vt prefetches ride the sync queue, interleaved into the
            # weight stream so they can't race the phase-critical loads.
            qt_st = st_pool.tile([128, MC, RPC], bf16, tag="st")
            vt_st = st_pool.tile([128, MC, RPC], bf16, tag="st")

            def stage_chunk(st, src_d, g):
                nc.sync.dma_start(
                    st[:, 4 * g:4 * g + 4, :],
                    src_d.ap()[4 * g:4 * g + 4].rearrange("t p r -> p t r"))

            with ExitStack() as ph:
                ps_pool = ph.enter_context(
                    tc.tile_pool(name="kqps", bufs=4, space="PSUM"))
                with nc.named_scope("proj_k"):
                    for ct in range(MC):
                        wk_st = wk_tiles.pop(ct)
                        if ct + 2 < MC:
                            wk_tiles[ct + 2] = load_w(w_pool,
                                                      wkt_d.ap()[ct + 2])
                        if 2 <= ct < 6:
                            stage_chunk(qt_st, qs_d, ct - 2)
                        for rb in range(2):
                            ps = ps_pool.tile([128, 4, 128], f32, tag="ps")
                            for mc in range(MC):
                                nc.tensor.matmul(
                                    ps[:], wk_st[:, mc, :],
                                    kt_st[:, mc, 512 * rb:512 * rb + 512],
                                    start=(mc == 0), stop=(mc == MC - 1))
                            nc.scalar.activation(
                                khat[:, 4 * rb:4 * rb + 4, ct, :], ps[:],
                                AF.Identity, bias=bk_sb[:, ct:ct + 1],
                                scale=1.0)

                # ============= phase Q =====================================
                with nc.named_scope("proj_q"):
                    for ct in range(MC):
                        wq_st = load_w(w_pool, wqt_d.ap()[ct])
                        if 2 <= ct < 6:
                            stage_chunk(vt_st, vs_d, ct - 2)
                        for rb in range(2):
                            ps = ps_pool.tile([128, 4, 128], f32, tag="ps")
                            for mc in range(MC):
                                nc.tensor.matmul(
                                    ps[:], wq_st[:, mc, :],
                                    qt_st[:, mc, 512 * rb:512 * rb + 512],
                                    start=(mc == 0), stop=(mc == MC - 1))
                            nc.scalar.activation(
                                qhat[:, ct, 512 * rb:512 * rb + 512],
                                ps[:].rearrange("p a u -> p (a u)"),
                                AF.Identity, bias=bq_sb[:, ct:ct + 1],
                                scale=1.0)

            # ============= phase V ========================================
            with ExitStack() as ph:
                ps_pool = ph.enter_context(
                    tc.tile_pool(name="vps", bufs=4, space="PSUM"))
                with nc.named_scope("proj_v"):
                    for cb in range(8):
                        wv_st = w_pool.tile([128, MC, 256], bf16, tag="w")
                        for g in range(2):
                            nc.sync.dma_start(
                                wv_st[:, 8 * g:8 * g + 8, :],
                                wvt_d.ap()[cb][:, 8 * g:8 * g + 8, :])
                        for rt in range(8):
                            ps = ps_pool.tile([128, 2, 128], f32, tag="ps")
                            for mc in range(MC):
                                nc.tensor.matmul(
                                    ps[:],
                                    vt_st[:, mc, 128 * rt:128 * rt + 128],
                                    wv_st[:, mc, :], start=(mc == 0),
                                    stop=False)
                            nc.tensor.matmul(
                                ps[:], ones1[:],
                                bv_sb[:, 256 * cb:256 * cb + 256],
                                start=False, stop=True)
                            nc.vector.tensor_copy(
                                vhat[:, rt, 2 * cb:2 * cb + 2, :], ps[:])

        # ============= attention + output projection ===================
        with ExitStack() as ph:
            wo_pool = ph.enter_context(tc.tile_pool(name="wo", bufs=1))
            ex_pool = ph.enter_context(tc.tile_pool(name="expp", bufs=6))
            tree_pool = ph.enter_context(tc.tile_pool(name="tree", bufs=2))
            x_pool = ph.enter_context(tc.tile_pool(name="xsb", bufs=2))
            nrm_pool = ph.enter_context(tc.tile_pool(name="nrm", bufs=2))
            oo_pool = ph.enter_context(tc.tile_pool(name="oout", bufs=3))
            sc_pool = ph.enter_context(
                tc.tile_pool(name="scps", bufs=2, space="PSUM"))
            xps_pool = ph.enter_context(
                tc.tile_pool(name="xps", bufs=2, space="PSUM"))
            sop_pool = ph.enter_context(
                tc.tile_pool(name="sop", bufs=2, space="PSUM"))

            # Wo tile loads here (first needed ~40us into attention).
            wo_sb = wo_pool.tile([128, MC, HPC, 128], bf16, tag="wo")
            for g in range(4):
                nc.gpsimd.dma_start(wo_sb[:, 4 * g:4 * g + 4, :, :],
                                    wot_d.ap()[:, 4 * g:4 * g + 4, :, :])

            with nc.named_scope("attn"):
                def attn_block(j, h, x_j, defer_in):
                    # q_rhs: direct SBUF view of qhat (no copy)
                    q_rhs = qhat[:, 4 * j:4 * j + 4, 128 * h:128 * h + 128]
                    x_ps = xps_pool.tile([128, 512], f32, tag="xps")
                    acc = tree_pool.tile([128, 2, 512], bf16, tag="acc")
                    tsum = tree_pool.tile([128, 512], bf16, tag="tsum")
                    exs = [None] * NP_
                    ys = [None] * 4
                    zs = [None] * 2

                    def pv_and_sum(tp):
                        ex = exs[tp]
                        for i in range(2):
                            nc.tensor.matmul(
                                x_ps[:], vhat[:, h, 2 * tp + i, :],
                                ex[:, i, :], start=(tp == 0 and i == 0),
                                stop=(tp == NP_ - 1 and i == 1))
                        if tp % 2 == 1:
                            y = tree_pool.tile([128, 2, 512], bf16, tag="y")
                            nc.vector.tensor_add(y[:], exs[tp - 1][:], ex[:])
                            ys[tp // 2] = y
                        if tp == 3:
                            z = tree_pool.tile([128, 2, 512], bf16, tag="z")
                            nc.vector.tensor_add(z[:], ys[0][:], ys[1][:])
                            zs[0] = z
                        elif tp == NP_ - 1:
                            z = tree_pool.tile([128, 2, 512], bf16, tag="z")
                            nc.vector.tensor_add(z[:], ys[2][:], ys[3][:])
                            zs[1] = z
                            nc.vector.tensor_add(acc[:], zs[0][:], zs[1][:])
                            nc.vector.tensor_add(tsum[:], acc[:, 0, :],
                                                 acc[:, 1, :])

                    def finisher():
                        s_ps = sop_pool.tile([128, 512], f32, tag="sop")
                        nc.tensor.matmul(s_ps[0:1, :], onescol[:], tsum[:],
                                         start=True, stop=True)
                        rec = nrm_pool.tile([1, 512], f32, tag="rec")
                        nc.vector.reciprocal_approx_fast(rec[:], s_ps[0:1, :])
                        bcast = nrm_pool.tile([128, 512], f32, tag="bc")
                        nc.gpsimd.partition_broadcast(bcast[:], rec[:])
                        nc.vector.tensor_mul(x_j[:, h, :], x_ps[:], bcast[:])

                    for tp in range(NP_):
                        sc = sc_pool.tile([128, 2, 512], f32, tag="sc")
                        for i in range(2):
                            nc.tensor.matmul(
                                sc[:, i, :], khat[:, h, 2 * tp + i, :],
                                q_rhs, start=True, stop=True)
                        ex = ex_pool.tile([128, 2, 512], bf16, tag="ex")
                        nc.scalar.activation(ex[:], sc[:], AF.Exp, scale=1.0)
                        exs[tp] = ex
                        if tp == 0:
                            for fn in defer_in:
                                fn()
                        if tp >= 2:
                            pv_and_sum(tp - 2)
                    return [lambda: pv_and_sum(NP_ - 2),
                            lambda: pv_and_sum(NP_ - 1), finisher]

                def emit_otile(j, x_j, ot):
                    op = sop_pool.tile([128, 512], f32, tag="sop")
                    for hh in range(HPC):
                        nc.tensor.matmul(op[:], wo_sb[:, ot, hh, :],
                                         x_j[:, hh, :], start=(hh == 0),
                                         stop=(hh == HPC - 1))
                    oo = oo_pool.tile([128, 512], bf16, tag="oo")
                    nc.vector.tensor_scalar_add(oo[:], op[:],
                                                bo_sb[:, ot:ot + 1])
                    nc.gpsimd.dma_start(
                        out_d.ap()[128 * ot:128 * ot + 128,
                                   512 * j:512 * j + 512], oo[:])

                # emit queue runs one j-block behind; the last block keeps two
                # emits in reserve to cover the final normalize chain.
                emitq = []
                defer = []
                for j in range(4):
                    x_j = x_pool.tile([128, HPC, 512], bf16, tag="xj")
                    for h in range(HPC):
                        defer = attn_block(j, h, x_j, defer)
                        npop = 0 if (j == 3 and h == 3) else 2
                        for _ in range(npop):
                            if emitq:
                                emit_otile(*emitq.pop(0))
                    emitq += [(j, x_j, ot) for ot in range(MC)]
                for fn in defer:
                    fn()
                for e in emitq:
                    emit_otile(*e)

    nc.compile()
    return nc


def _prep_shared(Wq, Wk, Wv, bq, bk, bv):
    import ml_dtypes
    bf = ml_dtypes.bfloat16
    wqt = (np.asarray(Wq, np.float32).T * SCALE).astype(bf)
    wkt = np.asarray(Wk, np.float32).T.astype(bf)
    wvt = np.asarray(Wv, np.float32).T.astype(bf)
    wqt_t = np.ascontiguousarray(
        wqt.reshape(MC, 128, MC, 128).transpose(2, 1, 0, 3))
    wkt_t = np.ascontiguousarray(
        wkt.reshape(MC, 128, MC, 128).transpose(2, 1, 0, 3))
    wvt_t = np.ascontiguousarray(
        wvt.reshape(MC, 128, 8, 256).transpose(2, 1, 0, 3))
    bqs = (np.asarray(bq, np.float32) * SCALE).copy()
    bk_np = np.asarray(bk, np.float32).copy()
    bvr = np.asarray(bv, np.float32).reshape(1, D).astype(bf)
    return wqt_t, wkt_t, wvt_t, bqs, bk_np, bvr


def kernel(Q, K, V, Wq, bq, Wk, bk, Wv, bv, Wo, bo, num_heads):
    global last_results
    assert int(num_heads) == H

    import ml_dtypes
    from concourse.bass_utils import run_bass_kernel_spmd

    bf = ml_dtypes.bfloat16

    if "nc" not in _cache:
        _cache["nc"] = _build()
    nc = _cache["nc"]

    Q = np.asarray(Q, np.float32)
    K = np.asarray(K, np.float32)
    V = np.asarray(V, np.float32)
    wqt_t, wkt_t, wvt_t, bqs, bk_np, bvr = _prep_shared(Wq, Wk, Wv, bq, bk, bv)
    woT = np.ascontiguousarray(np.asarray(Wo, np.float32).T)
    bo_np = np.asarray(bo, np.float32).copy()
    bo_zero = np.zeros_like(bo_np)
    ones1 = np.ones((1, 128), bf)
    onescol = np.ones((128, 1), bf)

    in_maps = []
    for c in range(NC_):
        b, half = divmod(c, 2)
        r0 = RPC * half
        wot_t = np.ascontiguousarray(
            woT[r0:r0 + RPC, :].astype(bf)
            .reshape(HPC, 128, MC, 128).transpose(1, 2, 0, 3))
        in_maps.append({
            "qs": Q[b].T[:, r0:r0 + RPC].astype(bf).reshape(MC, 128, RPC),
            "ks": K[b].T[:, r0:r0 + RPC].astype(bf).reshape(MC, 128, RPC),
            "vs": V[b].T[:, r0:r0 + RPC].astype(bf).reshape(MC, 128, RPC),
            "wqt": wqt_t, "wkt": wkt_t, "wvt": wvt_t, "wot": wot_t,
            "bqs": bqs, "bk": bk_np, "bvr": bvr,
            "bo": bo_np if half == 0 else bo_zero,
            "ones1": ones1, "onescol": onescol,
        })

    res = run_bass_kernel_spmd(nc, in_maps, core_ids=list(range(NC_)))
    last_results = res

    out = np.empty((B, S, D), np.float32)
    for b in range(B):
        oT = (res.results[2 * b]["out"].astype(np.float32)
              + res.results[2 * b + 1]["out"].astype(np.float32))
        # oT[o, pi], pi = 128*t + u ; s = 16*u + t
        out[b] = oT.reshape(D, 16, 128).transpose(2, 1, 0).reshape(S, D)
    return out
